# revision 1
# baseline (speedup 1.0000x reference)
"""Causal attention kernel for 8 Trainium2 NeuronCores.

Problem: x[4, 4096, 512] @ {Wq,Wk,Wv}[512, 128] -> causal attention -> [4, 4096, 128].

Sharding: 2 cores per batch, interleaved over KEY chunks. Core c = 2b+p
(batch b, parity p) owns key chunks {2i+p : i=0..15} (chunks of 128 keys),
and computes, for every query block of its batch, the partial softmax
numerator and denominator over its keys. The host sums the two partials and
divides. Causality makes query block qg (512 rows) attend key chunks
0..4qg+3, of which each parity owns exactly 2qg+2 -> both cores run the
identical program (exact load balance); only the last two local chunks of
each block need a (parity-dependent, input-supplied) additive mask.

On-device layout: scores are computed transposed, S^T[key, q]:
  - K^T[d, key], Q^T[d, q] come from host-pre-transposed x (no on-device
    transposes anywhere); 1/sqrt(d) is folded into Wq on the host
  - S^T chunk = matmul(lhsT=K^T[:, chunk], rhs=Q^T[:, qblock])  [N=512]
  - E = exp(S^T + mask) via ScalarE, straight out of PSUM
  - numerator^T[e, q] accumulates in PSUM: matmul(lhsT=V[chunk], rhs=E)
  - denominator[1, q] accumulates in PSUM: matmul(lhsT=ones, rhs=E)

MODE selects matmul operand precision (PSUM accumulation is always fp32):
  "bf16" - operands bf16 (x/W cast on host, halving input DMA); fastest
  "f32r" - single-pass fp32 matmul, ~12-bit mantissa operands
  "f32"  - exact fp32 (2-pass LOW_HIGH matmuls), slowest
"""

import math

import numpy as np

B, S, DIN, DOUT = 4, 4096, 512, 128
NCORES = 8
TQ = 512            # query block size
NQB = S // TQ       # 8 query blocks per batch
KC = 128            # key chunk size
NKLOC = S // KC // 2  # 16 key chunks owned per core
NEG = -1.0e9

MODE = "bf16"

_cache = {}


def _np_in_dtype(mode):
    if mode == "bf16":
        import ml_dtypes

        return ml_dtypes.bfloat16
    return np.float32


def _build_nc(mode=None):
    import concourse.bacc as bacc
    import concourse.mybir as mybir
    import concourse.tile as tile

    mode = MODE if mode is None else mode
    f32 = mybir.dt.float32
    mdt = {
        "f32": f32,
        "f32r": mybir.dt.float32r,
        "bf16": mybir.dt.bfloat16,
    }[mode]

    nc = bacc.Bacc(None, target_bir_lowering=False, debug=False)

    NDC = DIN // 128    # 4 contraction chunks for the projections
    SK = NKLOC * KC     # 2048 owned keys
    WMC = 3 * NDC * DOUT + 2 * TQ  # packed w+masks columns

    # All inputs are host-packed partition-major so every DMA is contiguous:
    # xb[p, c, s] = x[b].T[c*128+p, s], wm[p, :] = [wq|wk|wv chunks, masks]
    xb_d = nc.declare_dram_parameter("xb", [128, NDC, S], mdt, isOutput=False)
    xk_d = nc.declare_dram_parameter("xk", [128, NDC, SK], mdt, isOutput=False)
    wm_d = nc.declare_dram_parameter("wm", [128, WMC], mdt, isOutput=False)
    numT = nc.declare_dram_parameter("numT", [DOUT, S], f32, isOutput=True)
    den = nc.declare_dram_parameter("den", [NQB, TQ], f32, isOutput=True)


    with tile.TileContext(nc) as tc:
        with (
            tc.tile_pool(name="persist", bufs=1) as persist,
            tc.tile_pool(name="pss", bufs=5, space="PSUM") as pss,
            tc.tile_pool(name="pso", bufs=2, space="PSUM") as pso,
            tc.tile_pool(name="psd", bufs=1, space="PSUM") as psd,
            tc.tile_pool(name="etile", bufs=10) as etile,
            tc.tile_pool(name="otile", bufs=2) as otile,
            tc.tile_pool(name="dtile", bufs=2) as dtile,
        ):
            # ---- resident SBUF tensors ----
            xb_t = persist.tile([128, NDC, S], mdt, tag="xb")
            xk_t = persist.tile([128, NDC, SK], mdt, tag="xk")
            wm_t = persist.tile([128, WMC], mdt, tag="wm")
            qT = persist.tile([128, S], mdt, tag="qT")
            kT = persist.tile([128, SK], mdt, tag="kT")
            v_t = persist.tile([128, NKLOC, DOUT], mdt, tag="v")

            def w_ap(wi, c):
                return wm_t[:, (wi * NDC + c) * DOUT:(wi * NDC + c + 1) * DOUT]

            def mask_ap(m):
                return wm_t[:, 3 * NDC * DOUT + m * TQ:3 * NDC * DOUT + (m + 1) * TQ]

            # Input DMA: issue order matters (each HWDGE ring is FIFO and a
            # dma_start occupies the ring ~0.6us regardless of size), so use
            # few, large DMAs, most-urgent first. sync ring: K/V-path inputs;
            # scalar ring: Q-path inputs. rearrange folds the DIN chunking
            # into a single access pattern.
            ones_f = persist.tile([128, 1], f32, tag="ones_f")
            nc.vector.memset(ones_f, 1.0)
            ones = persist.tile([128, 1], mdt, tag="ones")
            nc.vector.tensor_copy(ones[:], ones_f[:])

            nc.sync.dma_start(out=wm_t[:], in_=wm_d[:])
            # xk: small lead piece so K-proj slice 0 starts early, then halves
            for sl in (slice(0, 512), slice(512, SK // 2), slice(SK // 2, SK)):
                nc.sync.dma_start(out=xk_t[:, :, sl], in_=xk_d[:, :, sl])
            # xb on the scalar ring, progressive for Q-proj
            for sl in (
                slice(0, 512),
                slice(512, 1024),
                slice(1024, 2048),
                slice(2048, 3072),
                slice(3072, S),
            ):
                nc.scalar.dma_start(out=xb_t[:, :, sl], in_=xb_d[:, :, sl])

            # ---- projections (K/V first: attention consumes them earliest) ----
            for s512 in range(SK // 512):  # K^T over owned keys
                ps = pss.tile([128, 512], f32, tag="ps_s", name=f"psk{s512}")
                for c in range(NDC):
                    nc.tensor.matmul(
                        ps[:],
                        w_ap(1, c),
                        xk_t[:, c, s512 * 512:(s512 + 1) * 512],
                        start=(c == 0),
                        stop=(c == NDC - 1),
                    )
                nc.vector.tensor_copy(kT[:, s512 * 512:(s512 + 1) * 512], ps[:])
            for t in range(NKLOC):  # V[key, e] natural layout, owned keys
                ps = pss.tile([128, 512], f32, tag="ps_s", name=f"psv{t}")
                for c in range(NDC):
                    nc.tensor.matmul(
                        ps[:, :DOUT],
                        xk_t[:, c, t * KC:(t + 1) * KC],
                        w_ap(2, c),
                        start=(c == 0),
                        stop=(c == NDC - 1),
                    )
                nc.vector.tensor_copy(v_t[:, t, :], ps[:, :DOUT])
            for s512 in range(S // 512):  # Q^T over all queries, in block order
                ps = pss.tile([128, 512], f32, tag="ps_s", name=f"psq{s512}")
                for c in range(NDC):
                    nc.tensor.matmul(
                        ps[:],
                        w_ap(0, c),
                        xb_t[:, c, s512 * 512:(s512 + 1) * 512],
                        start=(c == 0),
                        stop=(c == NDC - 1),
                    )
                nc.vector.tensor_copy(qT[:, s512 * 512:(s512 + 1) * 512], ps[:])

            # ---- attention ----
            for qg in range(NQB):
                n_loc = 2 * qg + 2
                po = pso.tile([128, TQ], f32, tag="po", name=f"po{qg}")
                pd = psd.tile([1, TQ], f32, tag="pd", name=f"pd{qg}")
                for i in range(n_loc):
                    ps = pss.tile([128, TQ], f32, tag="ps_s", name=f"pss{qg}_{i}")
                    masked = i >= n_loc - 2
                    if masked:
                        # pre-bias PSUM with the additive causal mask (off the
                        # critical path), then accumulate scores onto it
                        nc.vector.tensor_copy(ps[:], mask_ap(i - (n_loc - 2)))
                    nc.tensor.matmul(
                        ps[:],
                        kT[:, i * KC:(i + 1) * KC],
                        qT[:, qg * TQ:(qg + 1) * TQ],
                        start=not masked,
                        stop=True,
                    )
                    e = etile.tile([128, TQ], mdt, tag="e", name=f"e{qg}_{i}")
                    nc.scalar.activation(
                        e[:], ps[:], mybir.ActivationFunctionType.Exp
                    )
                    nc.tensor.matmul(
                        po[:],
                        v_t[:, i, :],
                        e[:],
                        start=(i == 0),
                        stop=(i == n_loc - 1),
                    )
                    nc.tensor.matmul(
                        pd[:],
                        ones[:],
                        e[:],
                        start=(i == 0),
                        stop=(i == n_loc - 1),
                    )
                o = otile.tile([128, TQ], f32, tag="o", name=f"o{qg}")
                nc.vector.tensor_copy(o[:], po[:])
                nc.scalar.dma_start(out=numT[:, qg * TQ:(qg + 1) * TQ], in_=o[:])
                d = dtile.tile([1, TQ], f32, tag="d", name=f"d{qg}")
                nc.vector.tensor_copy(d[:], pd[:])
                nc.sync.dma_start(out=den[qg:qg + 1, :], in_=d[:])

    nc.finalize()
    return nc


def _owned_keys(par):
    return np.concatenate(
        [np.arange((2 * i + par) * KC, (2 * i + par) * KC + KC) for i in range(NKLOC)]
    )


def _build_masks(par):
    # last two local chunks of each query block: relative chunk r0 = par,
    # r1 = 2 + par; element [k, q] allowed iff 128*r + k <= q
    r = np.array([par, 2 + par])[:, None, None]
    k = np.arange(KC)[None, :, None]
    q = np.arange(TQ)[None, None, :]
    allowed = (KC * r + k) <= q
    return np.where(allowed, np.float32(0.0), np.float32(NEG)).astype(np.float32)


def _get_nc():
    if "nc" not in _cache:
        _cache["nc"] = _build_nc()
    return _cache["nc"]


def _pack_pm(a):
    # [DIN, cols] -> partition-major [128, DIN//128, cols]
    return np.ascontiguousarray(a.reshape(DIN // 128, 128, a.shape[1]).transpose(1, 0, 2))


def _prepare_in_maps(x, Wq, Wk, Wv, mode=None):
    mode = MODE if mode is None else mode
    idt = _np_in_dtype(mode)
    ws = [(Wq / math.sqrt(DOUT)).astype(idt), Wk.astype(idt), Wv.astype(idt)]
    w_pack = np.concatenate(
        [_pack_pm(w).reshape(128, -1) for w in ws], axis=1
    )  # [128, 1536]
    in_maps = []
    for c in range(NCORES):
        b, par = c // 2, c % 2
        xbt = x[b].T.astype(idt)
        m = _build_masks(par).astype(idt)  # [2, 128, 512]
        wm = np.concatenate(
            [w_pack, np.ascontiguousarray(m.transpose(1, 0, 2)).reshape(128, -1)],
            axis=1,
        )
        in_maps.append({
            "xb": _pack_pm(xbt),
            "xk": _pack_pm(np.ascontiguousarray(xbt[:, _owned_keys(par)])),
            "wm": np.ascontiguousarray(wm),
        })
    return in_maps


def _gather(results):
    out = np.empty((B, S, DOUT), dtype=np.float32)
    for b in range(B):
        r0, r1 = results[2 * b], results[2 * b + 1]
        num = r0["numT"].astype(np.float64).T + r1["numT"].astype(np.float64).T
        d = r0["den"].astype(np.float64).reshape(-1) + r1["den"].astype(
            np.float64
        ).reshape(-1)
        out[b] = (num / d[:, None]).astype(np.float32)
    return out


def kernel(**inputs):
    from concourse.bass_utils import run_bass_kernel_spmd

    x = np.asarray(inputs["x"], dtype=np.float32)
    Wq = np.asarray(inputs["Wq"], dtype=np.float32)
    Wk = np.asarray(inputs["Wk"], dtype=np.float32)
    Wv = np.asarray(inputs["Wv"], dtype=np.float32)

    nc = _get_nc()
    in_maps = _prepare_in_maps(x, Wq, Wk, Wv)
    res = run_bass_kernel_spmd(nc, in_maps, list(range(NCORES)))
    return _gather(res.results)



# revision 2
# speedup vs baseline: 1.2411x; 1.2411x over previous
"""Causal attention kernel for 8 Trainium2 NeuronCores.

Problem: x[4, 4096, 512] @ {Wq,Wk,Wv}[512, 128] -> causal attention -> [4, 4096, 128].

Sharding: 2 cores per batch, interleaved over KEY chunks. Core c = 2b+p
(batch b, parity p) owns key chunks {2i+p : i=0..15} (chunks of 128 keys),
and computes, for every query block of its batch, the partial softmax
numerator and denominator over its keys. The host sums the two partials and
divides. Causality makes query block qg (512 rows) attend key chunks
0..4qg+3, of which each parity owns exactly 2qg+2 -> both cores run the
identical program (exact load balance); only the last two local chunks of
each block are masked (multiplicative {0,1} mask, input-supplied).

On-device layout: scores are computed transposed, S^T[key, q]:
  - projections run in fp8e4 DoubleRow (2 contraction tiles per pass),
    outputs cast to bf16 (K^T, Q^T) / fp8 (V)
  - S^T chunk = bf16 matmul(lhsT=K^T[:, chunk], rhs=Q^T[:, qblock])
  - chunks are processed in PAIRS: both score matmuls land in one 2-bank
    PSUM tile; ONE ScalarE exp (scale=1/sqrt(d)) covers the pair,
    amortizing the fixed activation overhead; output E is fp8
  - the causal mask is {0,1}-multiplicative, applied post-exp on DVE to
    the pair's E tile (only the last pair of each block)
  - numerator: ONE fp8 DoubleRow matmul per pair (lhsT = V chunk pair)
  - denominator: ONE fp8 DoubleRow matmul per pair with a one-hot
    stationary [128, 2, 8] that routes the sum into row qg of a single
    persistent PSUM bank [8, 512] holding all 8 blocks' denominators
  - query block 0 (rows with few attended keys) runs E/V in bf16 to keep
    fp8 quantization out of the near-copy early rows; everything else
    tolerates fp8 (softmax-weight averaging suppresses the error)
"""

import math

import numpy as np

B, S, DIN, DOUT = 4, 4096, 512, 128
NCORES = 8
TQ = 512            # query block size
NQB = S // TQ       # 8 query blocks per batch
KC = 128            # key chunk size
NKLOC = S // KC // 2  # 16 key chunks owned per core
SK = NKLOC * KC     # 2048 owned keys
NDC = DIN // 128    # 4 contraction chunks
RSQRT_D = 1.0 / math.sqrt(float(DOUT))

_cache = {}


def _build_nc():
    import concourse.bacc as bacc
    import concourse.mybir as mybir
    import concourse.tile as tile

    f32 = mybir.dt.float32
    bf = mybir.dt.bfloat16
    f8 = mybir.dt.float8e4
    DR = mybir.MatmulPerfMode.DoubleRow
    EXP = mybir.ActivationFunctionType.Exp
    MUL = mybir.AluOpType.mult

    nc = bacc.Bacc(None, target_bir_lowering=False, debug=False)

    # ---- DRAM parameters ----
    xq8_d = nc.declare_dram_parameter("xq8", [128, NDC, S], f8, isOutput=False)
    xk8_d = nc.declare_dram_parameter("xk8", [128, NDC, SK], f8, isOutput=False)
    xv16_d = nc.declare_dram_parameter("xv16", [128, NDC, 2 * KC], bf, isOutput=False)
    w8_d = nc.declare_dram_parameter("w8", [128, 3, NDC, DOUT], f8, isOutput=False)
    wv16_d = nc.declare_dram_parameter("wv16", [128, NDC, DOUT], bf, isOutput=False)
    mk8_d = nc.declare_dram_parameter("mk8", [128, 2, TQ], f8, isOutput=False)
    oh8_d = nc.declare_dram_parameter("oh8", [128, 2, 8 * NQB], f8, isOutput=False)
    oh16_d = nc.declare_dram_parameter("oh16", [128, 8], bf, isOutput=False)
    numT = nc.declare_dram_parameter("numT", [DOUT, S], f32, isOutput=True)
    den = nc.declare_dram_parameter("den", [NQB, TQ], f32, isOutput=True)

    with tile.TileContext(nc) as tc:
        with (
            tc.tile_pool(name="persist", bufs=1) as persist,
            tc.tile_pool(name="pp", bufs=1, space="PSUM") as pp,
            tc.tile_pool(name="ps2", bufs=2, space="PSUM") as ps2,
            tc.tile_pool(name="pso", bufs=2, space="PSUM") as pso,
            tc.tile_pool(name="psd", bufs=1, space="PSUM") as psd,
            tc.tile_pool(name="et", bufs=4) as et,
            tc.tile_pool(name="ot", bufs=2) as ot,
            tc.tile_pool(name="dt", bufs=1) as dt_pool,
        ):
            # ---- resident SBUF tensors ----
            xq8_t = persist.tile([128, NDC, S], f8, tag="xq8")
            xk8_t = persist.tile([128, NDC, SK], f8, tag="xk8")
            xv16_t = persist.tile([128, NDC, 2 * KC], bf, tag="xv16")
            w8_t = persist.tile([128, 3, NDC, DOUT], f8, tag="w8")
            wv16_t = persist.tile([128, NDC, DOUT], bf, tag="wv16")
            mk8_t = persist.tile([128, 2, TQ], f8, tag="mk8")
            oh8_t = persist.tile([128, 2, 8 * NQB], f8, tag="oh8")
            oh16_t = persist.tile([128, 8], bf, tag="oh16")
            qT = persist.tile([128, S], bf, tag="qT")
            kT = persist.tile([128, SK], bf, tag="kT")
            v8_t = persist.tile([128, NKLOC, DOUT], f8, tag="v8")
            v16_t = persist.tile([128, 2, DOUT], bf, tag="v16")

            # ---- input DMA: weights first, then lead pieces so the first
            # projections start early. Critical-path loads on the sync
            # (SP) HWDGE ring; the rest from the idle gpsimd queue.
            nc.sync.dma_start(out=w8_t[:], in_=w8_d[:])
            nc.sync.dma_start(out=xk8_t[:, :, 0:512], in_=xk8_d[:, :, 0:512])
            nc.sync.dma_start(out=xq8_t[:, :, 0:512], in_=xq8_d[:, :, 0:512])
            nc.gpsimd.dma_start(out=xv16_t[:], in_=xv16_d[:])
            nc.gpsimd.dma_start(out=wv16_t[:], in_=wv16_d[:])
            nc.gpsimd.dma_start(out=mk8_t[:], in_=mk8_d[:])
            nc.gpsimd.dma_start(out=oh8_t[:], in_=oh8_d[:])
            nc.gpsimd.dma_start(out=oh16_t[:], in_=oh16_d[:])
            nc.sync.dma_start(out=xk8_t[:, :, 512:SK], in_=xk8_d[:, :, 512:SK])
            nc.sync.dma_start(out=xq8_t[:, :, 512:2048], in_=xq8_d[:, :, 512:2048])
            nc.sync.dma_start(out=xq8_t[:, :, 2048:S], in_=xq8_d[:, :, 2048:S])

            def kproj(g):  # K^T for owned keys [512g, 512g+512)
                ps = pp.tile([128, 512], f32, tag="pp", name=f"ppk{g}")
                for j in (0, 1):
                    nc.tensor.matmul(
                        ps[:],
                        w8_t[:, 1, 2 * j:2 * j + 2, :],
                        xk8_t[:, 2 * j:2 * j + 2, 512 * g:512 * (g + 1)],
                        start=(j == 0),
                        stop=(j == 1),
                        perf_mode=DR,
                    )
                nc.vector.tensor_copy(kT[:, 512 * g:512 * (g + 1)], ps[:])

            def qproj(g):  # Q^T for queries [512g, 512g+512)
                ps = pp.tile([128, 512], f32, tag="pp", name=f"ppq{g}")
                for j in (0, 1):
                    nc.tensor.matmul(
                        ps[:],
                        w8_t[:, 0, 2 * j:2 * j + 2, :],
                        xq8_t[:, 2 * j:2 * j + 2, 512 * g:512 * (g + 1)],
                        start=(j == 0),
                        stop=(j == 1),
                        perf_mode=DR,
                    )
                nc.vector.tensor_copy(qT[:, 512 * g:512 * (g + 1)], ps[:])

            def vproj(g):  # V for local chunks 4g..4g+3, fp8
                ps = pp.tile([128, 4, DOUT], f32, tag="pp", name=f"ppv{g}")
                for c in range(4):
                    ck = 4 * g + c
                    for j in (0, 1):
                        nc.tensor.matmul(
                            ps[:, c, :],
                            xk8_t[:, 2 * j:2 * j + 2, KC * ck:KC * (ck + 1)],
                            w8_t[:, 2, 2 * j:2 * j + 2, :],
                            start=(j == 0),
                            stop=(j == 1),
                            perf_mode=DR,
                        )
                nc.vector.tensor_copy(v8_t[:, 4 * g:4 * (g + 1), :], ps[:])

            def v16proj():  # bf16 V for local chunks 0,1 (block-0 accuracy)
                ps = pp.tile([128, 2, DOUT], f32, tag="pp", name="ppv16")
                for c in (0, 1):
                    for t in range(NDC):
                        nc.tensor.matmul(
                            ps[:, c, :],
                            xv16_t[:, t, KC * c:KC * (c + 1)],
                            wv16_t[:, t, :],
                            start=(t == 0),
                            stop=(t == NDC - 1),
                        )
                nc.vector.tensor_copy(v16_t[:], ps[:])

            pd = psd.tile([8, TQ], f32, tag="pd", name="pd")

            def attn_block(qg):
                npairs = qg + 1
                po = pso.tile([128, TQ], f32, tag="po", name=f"po{qg}")
                for i in range(npairs):
                    pair = ps2.tile([128, 2, TQ], f32, tag="ps2", name=f"ps{qg}_{i}")
                    for c in (0, 1):
                        ck = 2 * i + c
                        nc.tensor.matmul(
                            pair[:, c, :],
                            kT[:, KC * ck:KC * (ck + 1)],
                            qT[:, TQ * qg:TQ * (qg + 1)],
                            start=True,
                            stop=True,
                        )
                    edt = bf if qg == 0 else f8
                    etag = "e16" if qg == 0 else "e8"
                    e = et.tile([128, 2, TQ], edt, tag=etag, name=f"e{qg}_{i}")
                    nc.scalar.activation(e[:], pair[:], EXP, scale=RSQRT_D)
                    if i == npairs - 1:
                        nc.vector.tensor_tensor(e[:], e[:], mk8_t[:], op=MUL)
                    last_den = qg == NQB - 1 and i == npairs - 1
                    if qg == 0:
                        for c in (0, 1):
                            nc.tensor.matmul(
                                po[:],
                                v16_t[:, c, :],
                                e[:, c, :],
                                start=(c == 0),
                                stop=(c == 1),
                            )
                            nc.tensor.matmul(
                                pd[:],
                                oh16_t[:],
                                e[:, c, :],
                                start=(c == 0),
                                stop=False,
                                skip_group_check=True,
                            )
                    else:
                        nc.tensor.matmul(
                            po[:],
                            v8_t[:, 2 * i:2 * i + 2, :],
                            e[:],
                            start=(i == 0),
                            stop=(i == npairs - 1),
                            perf_mode=DR,
                        )
                        nc.tensor.matmul(
                            pd[:],
                            oh8_t[:, :, 8 * qg:8 * (qg + 1)],
                            e[:],
                            start=False,
                            stop=last_den,
                            perf_mode=DR,
                            skip_group_check=True,
                        )
                o = ot.tile([128, TQ], f32, tag="o", name=f"o{qg}")
                nc.vector.tensor_copy(o[:], po[:])
                nc.gpsimd.dma_start(out=numT[:, TQ * qg:TQ * (qg + 1)], in_=o[:])

            # ---- schedule: projections interleaved as tensor-engine filler
            kproj(0)
            v16proj()
            vproj(0)
            qproj(0)
            attn_block(0)
            kproj(1)
            vproj(1)
            qproj(1)
            attn_block(1)
            kproj(2)
            vproj(2)
            qproj(2)
            attn_block(2)
            kproj(3)
            vproj(3)
            qproj(3)
            attn_block(3)
            for qg in range(4, NQB):
                qproj(qg)
                attn_block(qg)

            d = dt_pool.tile([8, TQ], f32, tag="d", name="d")
            nc.vector.tensor_copy(d[:], pd[:])
            nc.gpsimd.dma_start(out=den[:, :], in_=d[:])

    nc.finalize()
    return nc


def _owned_keys(par):
    return np.concatenate(
        [np.arange((2 * i + par) * KC, (2 * i + par) * KC + KC) for i in range(NKLOC)]
    )


def _build_masks01(par):
    # multiplicative {0,1} masks for the last pair of each query block:
    # pair-half j in {0,1} is global chunk 4qg+2j+par; element [k, q]
    # allowed iff 128*(2j+par) + k <= q (same for every block)
    j = np.arange(2)[:, None, None]
    k = np.arange(KC)[None, :, None]
    q = np.arange(TQ)[None, None, :]
    allowed = (KC * (2 * j + par) + k) <= q
    return allowed.astype(np.float32)  # [2, 128, 512]


def _get_nc():
    if "nc" not in _cache:
        _cache["nc"] = _build_nc()
    return _cache["nc"]


def _pack_pm(a):
    # [DIN, cols] -> partition-major [128, DIN//128, cols]
    return np.ascontiguousarray(a.reshape(DIN // 128, 128, a.shape[1]).transpose(1, 0, 2))


def _prepare_in_maps(x, Wq, Wk, Wv):
    import ml_dtypes

    f8 = ml_dtypes.float8_e4m3
    bf = ml_dtypes.bfloat16

    # [128, 3, NDC, DOUT]: w8[p, i, c, e] = W_i[128c + p, e]
    w8 = np.stack([_pack_pm(w).reshape(128, NDC, DOUT) for w in (Wq, Wk, Wv)], axis=1)
    w8 = np.ascontiguousarray(w8).astype(f8)
    wv16 = _pack_pm(Wv).astype(bf)

    # one-hot denominator routers
    oh8 = np.zeros((128, 2, 8 * NQB), dtype=np.float32)
    for qg in range(NQB):
        oh8[:, :, 8 * qg + qg] = 1.0
    oh8 = oh8.astype(f8)
    oh16 = np.zeros((128, 8), dtype=np.float32)
    oh16[:, 0] = 1.0
    oh16 = oh16.astype(bf)

    in_maps = []
    for c in range(NCORES):
        b, par = c // 2, c % 2
        xbt = x[b].T.astype(np.float32)
        ok = _owned_keys(par)
        m = _build_masks01(par)  # [2, 128, 512]
        mk8 = np.ascontiguousarray(m.transpose(1, 0, 2)).astype(f8)
        in_maps.append({
            "xq8": _pack_pm(xbt).astype(f8),
            "xk8": _pack_pm(np.ascontiguousarray(xbt[:, ok])).astype(f8),
            "xv16": _pack_pm(np.ascontiguousarray(xbt[:, ok[:2 * KC]])).astype(bf),
            "w8": w8,
            "wv16": wv16,
            "mk8": mk8,
            "oh8": oh8,
            "oh16": oh16,
        })
    return in_maps


def _gather(results):
    out = np.empty((B, S, DOUT), dtype=np.float32)
    for b in range(B):
        r0, r1 = results[2 * b], results[2 * b + 1]
        num = r0["numT"].astype(np.float64).T + r1["numT"].astype(np.float64).T
        d = r0["den"].astype(np.float64).reshape(-1) + r1["den"].astype(
            np.float64
        ).reshape(-1)
        out[b] = (num / d[:, None]).astype(np.float32)
    return out


def kernel(**inputs):
    from concourse.bass_utils import run_bass_kernel_spmd

    x = np.asarray(inputs["x"], dtype=np.float32)
    Wq = np.asarray(inputs["Wq"], dtype=np.float32)
    Wk = np.asarray(inputs["Wk"], dtype=np.float32)
    Wv = np.asarray(inputs["Wv"], dtype=np.float32)

    nc = _get_nc()
    in_maps = _prepare_in_maps(x, Wq, Wk, Wv)
    res = run_bass_kernel_spmd(nc, in_maps, list(range(NCORES)))
    return _gather(res.results)
